# revision 9
# baseline (speedup 1.0000x reference)
"""Trainium2 Bass kernel: nearest-centroid (VQ codebook) assignment.

Reference math (per row n of x):  y[n] = argmin_k ||x_n - c_k||^2
                                      = argmin_k (||c_k||^2 - 2 x_n.c_k)
                                      = argmax_k (x_n.c_k - 0.5||c_k||^2)

Strategy (8 NeuronCores, data-parallel over the B*S=16384 rows):
  - host: transpose x-shard and centroids so the contraction dim (D=512)
    lands on SBUF partitions, and split each fp32 operand into fp16
    hi/lo pieces (x = xh + xl exactly to ~22 mantissa bits).
  - device, per core (2048 rows): for each 128-row tile and each half of
    the 4096 centroids, accumulate dots = xh.ch + xh.cl + xl.ch into
    PSUM with fp16 matmuls (full rate, exact products -> fp32-level
    accuracy; the dropped xl.cl term is ~1e-6 relative), subtract the
    0.5*||c||^2 bias row on DVE, then DVE max/max_index for the argmax;
    combine the two halves on-device and DMA one uint32 index per row.
"""

import sys

if "/opt/trn_rl_repo" not in sys.path:
    sys.path.insert(0, "/opt/trn_rl_repo")

import numpy as np

B, S, D, K = 8, 2048, 512, 4096
N_CORES = 8
N = B * S
N_PER_CORE = N // N_CORES          # 2048 rows per core
P = 128                            # SBUF partitions
MM_N = 512                         # matmul moving free dim (1 PSUM bank fp32)

_PROGRAM_CACHE = {}
LAST_RESULTS = None  # BassKernelResults of the most recent kernel() call


def _build_program(n_per_core=N_PER_CORE, k=K, d=D):
    import concourse.bacc as bacc
    import concourse.tile as tile
    from concourse import mybir

    f32 = mybir.dt.float32
    f16 = mybir.dt.float16
    u32 = mybir.dt.uint32

    DC = d // P                    # d-chunks (contraction tiles)
    ROW_TILES = n_per_core // P    # 128-row tiles per core
    NB = k // MM_N                 # psum banks needed for full k
    HB = NB // 2                   # banks per half
    HK = HB * MM_N                 # centroids per half

    nc = bacc.Bacc("TRN2", target_bir_lowering=False, debug=False)

    xh = nc.dram_tensor("xh", [d, n_per_core], f16, kind="ExternalInput").ap()
    xl = nc.dram_tensor("xl", [d, n_per_core], f16, kind="ExternalInput").ap()
    ch = nc.dram_tensor("ch", [d, k], f16, kind="ExternalInput").ap()
    cl = nc.dram_tensor("cl", [d, k], f16, kind="ExternalInput").ap()
    csqb = nc.dram_tensor("csqb", [P, k], f32, kind="ExternalInput").ap()
    y = nc.dram_tensor("y", [P, ROW_TILES], u32, kind="ExternalOutput").ap()

    with tile.TileContext(nc) as tc:
        with (
            tc.tile_pool(name="consts", bufs=1) as consts,
            tc.tile_pool(name="psum", bufs=2, space="PSUM") as psum_pool,
            tc.tile_pool(name="dists", bufs=3) as dists,
            tc.tile_pool(name="stage", bufs=1) as stage,
        ):
            xh_sb = consts.tile([P, DC * n_per_core], f16)
            xl_sb = consts.tile([P, DC * n_per_core], f16)
            ch_sb = consts.tile([P, DC * k], f16)
            cl_sb = consts.tile([P, DC * k], f16)
            csqb_sb = consts.tile([P, k], f32)

            # chunked loads, first-needed first (half 0 of the centroids,
            # the first x column block, the bias half 0), so compute can
            # start while the rest streams in.  chunk = 1024 columns.
            CHUNK = 1024
            emitted = set()

            def load_chunk(sb, dram, width, dc, c0):
                key = (id(sb), dc, c0)
                if key in emitted:
                    return
                emitted.add(key)
                cw = min(CHUNK, width - c0)
                nc.sync.dma_start(
                    out=sb[:, dc * width + c0 : dc * width + c0 + cw],
                    in_=dram[dc * P : (dc + 1) * P, c0 : c0 + cw],
                )

            def load_csqb(c0):
                if ("csqb", c0) in emitted:
                    return
                emitted.add(("csqb", c0))
                nc.sync.dma_start(
                    out=csqb_sb[:, c0 : c0 + CHUNK], in_=csqb[:, c0 : c0 + CHUNK]
                )

            # priority order
            for c0 in range(0, HK, CHUNK):
                for dc in range(DC):
                    load_chunk(ch_sb, ch, k, dc, c0)
                    load_chunk(cl_sb, cl, k, dc, c0)
            for dc in range(DC):
                load_chunk(xh_sb, xh, n_per_core, dc, 0)
                load_chunk(xl_sb, xl, n_per_core, dc, 0)
            for c0 in range(0, HK, CHUNK):
                load_csqb(c0)
            # the rest
            for c0 in range(HK, k, CHUNK):
                for dc in range(DC):
                    load_chunk(ch_sb, ch, k, dc, c0)
                    load_chunk(cl_sb, cl, k, dc, c0)
                load_csqb(c0)
            for c0 in range(CHUNK, n_per_core, CHUNK):
                for dc in range(DC):
                    load_chunk(xh_sb, xh, n_per_core, dc, c0)
                    load_chunk(xl_sb, xl, n_per_core, dc, c0)

            stA_mx = stage.tile([P, ROW_TILES * 8], f32)
            stB_mx = stage.tile([P, ROW_TILES * 8], f32)
            stA_ix = stage.tile([P, ROW_TILES * 8], u32)
            stB_ix = stage.tile([P, ROW_TILES * 8], u32)

            for t in range(ROW_TILES):
                for h in range(2):
                    ps = psum_pool.tile([P, HK], f32, name="ps", tag="ps")
                    # dots = xh.ch + xh.cl + xl.ch  (fp16 pieces, fp32 accum)
                    for dc in range(DC):
                        xh_t = xh_sb[:, dc * n_per_core + t * P : dc * n_per_core + (t + 1) * P]
                        xl_t = xl_sb[:, dc * n_per_core + t * P : dc * n_per_core + (t + 1) * P]
                        for pi, (lhs, rhs_sb) in enumerate(
                            ((xh_t, ch_sb), (xh_t, cl_sb), (xl_t, ch_sb))
                        ):
                            for b in range(HB):
                                nc.tensor.matmul(
                                    ps[:, b * MM_N : (b + 1) * MM_N],
                                    lhsT=lhs,
                                    rhs=rhs_sb[
                                        :,
                                        dc * k + h * HK + b * MM_N : dc * k
                                        + h * HK
                                        + (b + 1) * MM_N,
                                    ],
                                    start=(dc == 0 and pi == 0),
                                    stop=(dc == DC - 1 and pi == 2),
                                )
                    # m = dots - 0.5*||c||^2, then per-row argmax over the half
                    dist = dists.tile([P, HK], f32, name="dist", tag="dist")
                    nc.vector.tensor_tensor(
                        dist[:, :], ps[:, :], csqb_sb[:, h * HK : (h + 1) * HK],
                        mybir.AluOpType.subtract,
                    )
                    stmx = stA_mx if h == 0 else stB_mx
                    stix = stA_ix if h == 0 else stB_ix
                    nc.vector.max(stmx[:, t * 8 : t * 8 + 8], dist[:, :])
                    nc.vector.max_index(
                        stix[:, t * 8 : t * 8 + 8], stmx[:, t * 8 : t * 8 + 8], dist[:, :]
                    )

            # combine halves: winner value picks the half; ties go to half A
            # (the smaller index), matching argmin's first-occurrence rule.
            mxA0 = stA_mx.rearrange("p (t e) -> p e t", e=8)[:, 0, :]
            mxB0 = stB_mx.rearrange("p (t e) -> p e t", e=8)[:, 0, :]
            ixA0 = stA_ix.rearrange("p (t e) -> p e t", e=8)[:, 0, :]
            ixB0 = stB_ix.rearrange("p (t e) -> p e t", e=8)[:, 0, :]

            cond = stage.tile([P, ROW_TILES], mybir.dt.uint8)
            finu = stage.tile([P, ROW_TILES], u32)
            nc.vector.tensor_tensor(cond[:, :], mxA0, mxB0, mybir.AluOpType.is_ge)
            nc.vector.tensor_scalar(finu[:, :], ixB0, HK, None, mybir.AluOpType.add)
            nc.vector.copy_predicated(finu[:, :], cond[:, :], ixA0)
            nc.sync.dma_start(out=y, in_=finu[:, :])

    nc.compile()
    return nc


def _get_program():
    key = "full"
    if key not in _PROGRAM_CACHE:
        _PROGRAM_CACHE[key] = _build_program()
    return _PROGRAM_CACHE[key]


def _split_f16(a: np.ndarray) -> tuple[np.ndarray, np.ndarray]:
    """a ~= hi + lo with both pieces fp16; hi/lo products stay exact in
    fp32 matmul accumulation."""
    hi = a.astype(np.float16)
    lo = (a - hi.astype(np.float32)).astype(np.float16)
    return hi, lo


def kernel(x: np.ndarray, centroids: np.ndarray) -> tuple[np.ndarray, np.ndarray]:
    import os

    from concourse.bass_utils import run_bass_kernel_spmd

    x = np.asarray(x)
    centroids = np.asarray(centroids)
    assert x.shape == (B, S, D) and centroids.shape == (K, D)

    nc = _get_program()

    flat = np.ascontiguousarray(x.reshape(N, D).astype(np.float32, copy=False))
    ct_full = np.ascontiguousarray(centroids.T.astype(np.float32, copy=False))
    ch_full, cl_full = _split_f16(ct_full)
    csq = np.sum(centroids.astype(np.float32) * centroids.astype(np.float32), axis=-1)
    csqb_full = np.ascontiguousarray(
        np.broadcast_to((0.5 * csq).astype(np.float32)[None, :], (P, K))
    )

    in_maps = []
    for c in range(N_CORES):
        shard = np.ascontiguousarray(flat[c * N_PER_CORE : (c + 1) * N_PER_CORE].T)
        xh_c, xl_c = _split_f16(shard)
        in_maps.append(
            {
                "xh": xh_c,
                "xl": xl_c,
                "ch": ch_full,
                "cl": cl_full,
                "csqb": csqb_full,
            }
        )

    trace = bool(int(os.environ.get("KERNEL_TRACE", "0")))
    tmpdir = os.environ.get("KERNEL_TRACE_DIR") or None
    res = run_bass_kernel_spmd(
        nc, in_maps, list(range(N_CORES)), trace=trace, tmpdir=tmpdir
    )
    global LAST_RESULTS
    LAST_RESULTS = res

    parts = []
    for c in range(N_CORES):
        y_pt = res.results[c]["y"]            # [128, ROW_TILES], y[p, t] = row t*128+p
        parts.append(np.ascontiguousarray(y_pt.T).reshape(-1))
    y = np.concatenate(parts).view(np.int32).reshape(B, S)
    return x, y
